# revision 11
# baseline (speedup 1.0000x reference)
"""AttentionBlock (GroupNorm + 4-head self-attention + proj + residual) on 8 trn2 cores.

Sharding: data-parallel over batch (B=16 -> 2 per core). Each core runs the full
block on its 2 batch elements; no collectives.

Device pipeline per batch (all layouts chosen so no on-device transposes are needed):
  - GroupNorm stats via bn_stats/bn_aggr + tiny PE matmuls for the cross-partition
    group combine (gamma/beta folded into the qkv weights on host).
  - Q,K GEMM in [channel, spatial] layout; V^T via a swapped GEMM (stationary=h).
  - Scores computed transposed: S^T[m,n] = K^T Q, so softmax's sum runs over the
    PSUM partition axis, computed by an all-ones stationary matmul that also
    replicates the denominator Z across partitions (no broadcast needed).
  - exp on ACT straight out of PSUM; no max-subtraction (scores bounded ~|8|).
  - AV as O^T[d,n] = sum_m V^T[m,d] expS^T[m,n], 2 heads packed per 128-col array.
  - proj GEMM + residual; biases injected via rank-1 (K=1) matmuls into PSUM.
"""

import numpy as np
from contextlib import ExitStack

import concourse.bass as bass
import concourse.bacc as bacc
import concourse.tile as tile
import concourse.mybir as mybir
from concourse.bass_utils import run_bass_kernel_spmd

F32 = mybir.dt.float32
F32R = mybir.dt.float32r

B, C, HH, WW = 16, 256, 32, 32
N = HH * WW           # 1024 spatial positions
NH = 4                # heads
D = C // NH           # 64 head dim
G = 32                # groups
EPS = 1e-5
NCORES = 8
BL = B // NCORES      # batches per core

USE_F32R = True       # fp32r matmuls: full PE rate; flip to False for exact fp32
MM_DT = F32R if USE_F32R else F32


def _R(ap):
    return ap


def build_bass():
    nc = bacc.Bacc("TRN2", target_bir_lowering=False, debug=False)

    x_d = nc.dram_tensor("x", [BL, C, N], F32, kind="ExternalInput").ap()
    wqk_d = nc.dram_tensor("wqk_t", [2, 128, 512], MM_DT, kind="ExternalInput").ap()
    wv_d = nc.dram_tensor("wv_t", [2, 128, 256], MM_DT, kind="ExternalInput").ap()
    wp_d = nc.dram_tensor("wp_t", [2, 128, 256], MM_DT, kind="ExternalInput").ap()
    bqk_d = nc.dram_tensor("bqk", [4, 128], F32, kind="ExternalInput").ap()
    bv_d = nc.dram_tensor("bv", [1, 256], MM_DT, kind="ExternalInput").ap()
    bp_d = nc.dram_tensor("bp", [1, 256], MM_DT, kind="ExternalInput").ap()
    gmap_d = nc.dram_tensor("gmap", [128, 16], F32, kind="ExternalInput").ap()
    gexp_d = nc.dram_tensor("gexp", [16, 128], F32, kind="ExternalInput").ap()
    y_d = nc.dram_tensor("y", [BL, C, N], F32, kind="ExternalOutput").ap()

    Exp = mybir.ActivationFunctionType.Exp
    mult = mybir.AluOpType.mult
    sub = mybir.AluOpType.subtract
    add = mybir.AluOpType.add

    with tile.TileContext(nc) as tc, ExitStack() as ctx:
        consts = ctx.enter_context(tc.tile_pool(name="consts", bufs=1))
        xpool = ctx.enter_context(tc.tile_pool(name="xp", bufs=1))
        hpool = ctx.enter_context(tc.tile_pool(name="hp", bufs=1))
        qkpool = ctx.enter_context(tc.tile_pool(name="qkp", bufs=1))
        vtpool = ctx.enter_context(tc.tile_pool(name="vtp", bufs=1))
        opool = ctx.enter_context(tc.tile_pool(name="op", bufs=1))
        gnpool = ctx.enter_context(tc.tile_pool(name="gnp", bufs=2))
        expool = ctx.enter_context(tc.tile_pool(name="exp", bufs=3))
        rzpool = ctx.enter_context(tc.tile_pool(name="rzp", bufs=2))
        outpool = ctx.enter_context(tc.tile_pool(name="outp", bufs=2))
        ps_big = ctx.enter_context(tc.tile_pool(name="psb", bufs=2, space="PSUM"))
        ps_o = ctx.enter_context(tc.tile_pool(name="pso", bufs=3, space="PSUM"))
        ps_z = ctx.enter_context(tc.tile_pool(name="psz", bufs=1, space="PSUM"))

        # --- constants / weights into SBUF ---
        wqk_sb = [consts.tile([128, 512], MM_DT, tag=f"wqk{k}", name=f"wqk{k}") for k in range(2)]
        wv_sb = [consts.tile([128, 256], MM_DT, tag=f"wv{k}", name=f"wv{k}") for k in range(2)]
        wp_sb = [consts.tile([128, 256], MM_DT, tag=f"wp{k}", name=f"wp{k}") for k in range(2)]
        for k in range(2):
            nc.sync.dma_start(wqk_sb[k][:], wqk_d[k])
            nc.sync.dma_start(wv_sb[k][:], wv_d[k])
            nc.sync.dma_start(wp_sb[k][:], wp_d[k])
        bqk_sb = consts.tile([128, 4], F32, tag="bqk")
        nc.sync.dma_start(bqk_sb[:], bqk_d.transpose([1, 0]))
        bv_sb = consts.tile([1, 256], MM_DT, tag="bv")
        nc.sync.dma_start(bv_sb[:], bv_d[:])
        bp_sb = consts.tile([1, 256], MM_DT, tag="bp")
        nc.sync.dma_start(bp_sb[:], bp_d[:])
        gmap_sb = consts.tile([128, 16], F32, tag="gmap")
        nc.sync.dma_start(gmap_sb[:], gmap_d[:])
        gexp_sb = consts.tile([16, 128], F32, tag="gexp")
        nc.sync.dma_start(gexp_sb[:], gexp_d[:])
        ones_f32 = consts.tile([128, 512], F32, tag="ones_f32")
        nc.vector.memset(ones_f32[:], 1.0)
        ones_sb = consts.tile([128, 512], MM_DT, tag="ones")
        nc.vector.tensor_copy(ones_sb[:], ones_f32[:])

        x_sb = [[None] * 2 for _ in range(BL)]
        h_sb = [[None] * 2 for _ in range(BL)]
        qk_sb = [[None] * 4 for _ in range(BL)]
        vt_sb = [[None] * 8 for _ in range(BL)]
        o_sb = [[None] * 2 for _ in range(BL)]

        # ================= Phase A: GroupNorm + QKV GEMMs =================
        for b in range(BL):
            for ct in range(2):
                xt = xpool.tile([128, N], F32, tag=f"x{b}{ct}")
                x_sb[b][ct] = xt
                nc.sync.dma_start(xt[:], x_d[b, ct * 128:(ct + 1) * 128, :])

                # per-channel stats over the 1024 free elems
                bn6 = gnpool.tile([128, 12], F32, tag="bn6")
                nc.vector.bn_stats(bn6[:, 0:6], xt[:, 0:512])
                nc.vector.bn_stats(bn6[:, 6:12], xt[:, 512:1024])
                mv = gnpool.tile([128, 2], F32, tag="mv")  # mean, var per channel
                nc.vector.bn_aggr(mv[:], bn6[:])
                # st2 = [mean, E[x^2]] per channel
                st2 = gnpool.tile([128, 2], F32, tag="st2")
                nc.vector.tensor_copy(st2[:, 0:1], mv[:, 0:1])
                nc.vector.tensor_mul(st2[:, 1:2], mv[:, 0:1], mv[:, 0:1])
                nc.vector.tensor_add(st2[:, 1:2], st2[:, 1:2], mv[:, 1:2])
                # group-combine: [128,2] -> [16,2] (x 1/8) -> expand back [128,2]
                psg = ps_z.tile([16, 2], F32, tag="z")
                nc.tensor.matmul(psg[:], lhsT=gmap_sb[:], rhs=st2[:], start=True, stop=True)
                gs = gnpool.tile([16, 2], F32, tag="gs")
                nc.vector.tensor_copy(gs[:], psg[:])
                psc = ps_z.tile([128, 2], F32, tag="z")
                nc.tensor.matmul(psc[:], lhsT=gexp_sb[:], rhs=gs[:], start=True, stop=True)
                cs = gnpool.tile([128, 2], F32, tag="cs")  # [mean_g, E[x^2]_g]
                nc.vector.tensor_copy(cs[:], psc[:])
                # inv_std = sqrt(1/(var+eps)); var = E[x^2] - mean^2
                m2 = gnpool.tile([128, 1], F32, tag="m2")
                nc.vector.tensor_mul(m2[:], cs[:, 0:1], cs[:, 0:1])
                ve = gnpool.tile([128, 1], F32, tag="ve")
                nc.vector.tensor_sub(ve[:], cs[:, 1:2], m2[:])
                nc.vector.tensor_scalar_add(ve[:], ve[:], EPS)
                iv = gnpool.tile([128, 1], F32, tag="iv")
                nc.vector.reciprocal(iv[:], ve[:])
                s_t = gnpool.tile([128, 1], F32, tag="s_t")
                nc.scalar.sqrt(s_t[:], iv[:])
                t_t = gnpool.tile([128, 1], F32, tag="t_t")  # mean * inv_std
                nc.vector.tensor_mul(t_t[:], cs[:, 0:1], s_t[:])
                # h = x * s - t  (gamma/beta already folded into W/b on host)
                ht = hpool.tile([128, N], MM_DT, tag=f"h{b}{ct}")
                h_sb[b][ct] = ht
                nc.vector.tensor_scalar(ht[:], xt[:], s_t[:, 0:1], t_t[:, 0:1], mult, sub)

            # Q,K GEMM: out channels ot: 0=q01 1=q23 2=k01 3=k23 (K pre-scaled by 1/8)
            for ot in range(4):
                pq = ps_big.tile([128, N], F32, tag="big")
                for nch in range(2):
                    ns = slice(nch * 512, (nch + 1) * 512)
                    for k in range(2):
                        nc.tensor.matmul(
                            pq[:, ns],
                            lhsT=_R(wqk_sb[k][:, ot * 128:(ot + 1) * 128]),
                            rhs=_R(h_sb[b][k][:, ns]),
                            start=(k == 0), stop=(k == 1),
                        )
                qk = qkpool.tile([128, N], MM_DT, tag=f"qk{b}{ot}")
                qk_sb[b][ot] = qk
                nc.vector.tensor_scalar(qk[:], pq[:], bqk_sb[:, ot:ot + 1], None, add)

            # V^T GEMM: V^T[m, vc] = sum_c h[c,m] WvT[c,vc]  (+ bv via rank-1 matmul)
            for m in range(8):
                pv = ps_o.tile([128, 512], F32, tag="o")
                mc = slice(m * 128, (m + 1) * 128)
                for k in range(2):
                    nc.tensor.matmul(
                        pv[:, 0:256],
                        lhsT=_R(h_sb[b][k][:, mc]),
                        rhs=_R(wv_sb[k][:]),
                        start=(k == 0), stop=False,
                    )
                nc.tensor.matmul(
                    pv[:, 0:256],
                    lhsT=_R(ones_sb[0:1, 0:128]),
                    rhs=_R(bv_sb[0:1, :]),
                    start=False, stop=True,
                )
                # vt layout [128, 512]: per global head h a 128-col block;
                # even h: [V_h | ones], odd h: [ones | V_h].  The ones columns
                # make the AV matmul also emit the softmax denominator Z
                # (replicated over 64 partitions) in the same PSUM bank.
                vt = vtpool.tile([128, 512], MM_DT, tag=f"vt{b}{m}")
                vt_sb[b][m] = vt
                vt4 = vt[:].rearrange("p (a u v d) -> p a u v d", a=2, u=2, v=2)
                pv4 = pv[:, 0:256].rearrange("p (a w d) -> p a w d", a=2, w=2)
                of = ones_f32[:, 0:128].rearrange("p (a d) -> p a d", a=2)
                nc.vector.tensor_copy(vt4[:, :, 0, 0, :], pv4[:, :, 0, :])
                nc.vector.tensor_copy(vt4[:, :, 1, 1, :], pv4[:, :, 1, :])
                nc.vector.tensor_copy(vt4[:, :, 0, 1, :], of[:])
                nc.vector.tensor_copy(vt4[:, :, 1, 0, :], of[:])

        # ================= Phase B: attention, Phase C: proj+residual ======
        for b in range(BL):
            for p in range(2):
                ot = opool.tile([128, N], MM_DT, tag=f"o{b}{p}")
                o_sb[b][p] = ot
                qt = qk_sb[b][p]
                kt = qk_sb[b][2 + p]
                h0, h1 = 2 * p, 2 * p + 1
                for nch in range(2):
                    ns = slice(nch * 512, (nch + 1) * 512)
                    po0 = ps_o.tile([128, 512], F32, tag="o", name="po0")
                    po1 = ps_o.tile([128, 512], F32, tag="o", name="po1")
                    for m in range(8):
                        mc = slice(m * 128, (m + 1) * 128)
                        ps = ps_big.tile([128, N], F32, tag="big")
                        # S^T chunk: [m(128 part), n(512)] x 2 heads (row-packed)
                        nc.tensor.matmul(
                            ps[:, 0:512],
                            lhsT=_R(kt[0:64, mc]), rhs=_R(qt[0:64, ns]),
                            start=True, stop=True,
                        )
                        nc.tensor.matmul(
                            ps[:, 512:1024],
                            lhsT=_R(kt[64:128, mc]), rhs=_R(qt[64:128, ns]),
                            start=True, stop=True,
                        )
                        ex = expool.tile([128, N], MM_DT, tag="ex")
                        nc.scalar.activation(ex[:], ps[:], Exp)
                        first, last = (m == 0), (m == 7)
                        # AV+Z: [V_h0|1] -> O rows 0:64, Zrep rows 64:128
                        nc.tensor.matmul(
                            po0[:],
                            lhsT=_R(vt_sb[b][m][:, 128 * h0:128 * h0 + 128]),
                            rhs=_R(ex[:, 0:512]),
                            start=first, stop=last,
                        )
                        # [1|V_h1] -> Zrep rows 0:64, O rows 64:128
                        nc.tensor.matmul(
                            po1[:],
                            lhsT=_R(vt_sb[b][m][:, 128 * h1:128 * h1 + 128]),
                            rhs=_R(ex[:, 512:1024]),
                            start=first, stop=last,
                        )
                    # head h0: O at rows 0:64, 1/Z needs a partition shift down
                    rz0 = rzpool.tile([128, 512], F32, tag="rz", name="rz0")
                    nc.vector.reciprocal(rz0[64:128, :], po0[64:128, :])
                    rzs0 = rzpool.tile([64, 512], F32, tag="rzs0")
                    nc.sync.dma_start(rzs0[:], rz0[64:128, :])
                    nc.vector.tensor_mul(ot[0:64, ns], po0[0:64, :], rzs0[:])
                    # head h1: O at rows 64:128, 1/Z shifts up
                    rz1 = rzpool.tile([128, 512], F32, tag="rz", name="rz1")
                    nc.vector.reciprocal(rz1[0:64, :], po1[0:64, :])
                    rzs1 = rzpool.tile([128, 512], F32, tag="rzs1")
                    nc.sync.dma_start(rzs1[64:128, :], rz1[0:64, :])
                    nc.vector.tensor_mul(ot[64:128, ns], po1[64:128, :], rzs1[64:128, :])

            # proj + residual + bias
            for ct in range(2):
                pp = ps_big.tile([128, N], F32, tag="big")
                for nch in range(2):
                    ns = slice(nch * 512, (nch + 1) * 512)
                    for k in range(2):
                        nc.tensor.matmul(
                            pp[:, ns],
                            lhsT=_R(wp_sb[k][:, ct * 128:(ct + 1) * 128]),
                            rhs=_R(o_sb[b][k][:, ns]),
                            start=(k == 0), stop=False,
                        )
                    nc.tensor.matmul(
                        pp[:, ns],
                        lhsT=_R(bp_sb[0:1, ct * 128:(ct + 1) * 128]),
                        rhs=_R(ones_sb[0:1, :]),
                        start=False, stop=True,
                    )
                outt = outpool.tile([128, N], F32, tag="out")
                nc.vector.tensor_add(outt[:], pp[:], x_sb[b][ct][:])
                nc.sync.dma_start(y_d[b, ct * 128:(ct + 1) * 128, :], outt[:])

    nc.compile()
    return nc


def prep_inputs(x, gn_gamma, gn_beta, qkv_w, qkv_b, proj_w, proj_b):
    """Host-side weight prep shared by kernel() and the CoreSim test."""
    x = np.ascontiguousarray(np.asarray(x, np.float32)).reshape(B, C, N)
    gn_gamma = np.asarray(gn_gamma, np.float32)
    gn_beta = np.asarray(gn_beta, np.float32)
    qkv_w = np.asarray(qkv_w, np.float32)
    qkv_b = np.asarray(qkv_b, np.float32)
    proj_w = np.asarray(proj_w, np.float32)
    proj_b = np.asarray(proj_b, np.float32)

    # fold GroupNorm affine into the qkv GEMM
    W3 = qkv_w * gn_gamma[None, :]
    b3 = qkv_b + qkv_w @ gn_beta
    W3r = W3.reshape(NH, 3, D, C)
    b3r = b3.reshape(NH, 3, D)
    scale = np.float32(D ** -0.5)
    Wq = W3r[:, 0].reshape(C, C)
    Wk = W3r[:, 1].reshape(C, C) * scale   # fold the attention scale into K
    Wv = W3r[:, 2].reshape(C, C)
    bq = b3r[:, 0].reshape(C)
    bk = b3r[:, 1].reshape(C) * scale
    bv = b3r[:, 2].reshape(C)

    wqk_t = np.ascontiguousarray(
        np.concatenate([Wq, Wk], axis=0).T).reshape(2, 128, 512)
    wv_t = np.ascontiguousarray(Wv.T).reshape(2, 128, 256)
    wp_t = np.ascontiguousarray(proj_w.T).reshape(2, 128, 256)
    bqk = np.concatenate([bq, bk]).reshape(4, 128)

    cidx = np.arange(128)
    gmap = np.zeros((128, 16), np.float32)
    gmap[cidx, cidx // 8] = 1.0 / 8.0
    gexp = np.zeros((16, 128), np.float32)
    gexp[cidx // 8, cidx] = 1.0

    common = {
        "wqk_t": wqk_t.astype(np.float32),
        "wv_t": wv_t.astype(np.float32),
        "wp_t": wp_t.astype(np.float32),
        "bqk": bqk.astype(np.float32),
        "bv": np.ascontiguousarray(bv[None, :], np.float32),
        "bp": np.ascontiguousarray(proj_b[None, :], np.float32),
        "gmap": gmap,
        "gexp": gexp,
    }
    in_maps = [
        {**common, "x": np.ascontiguousarray(x[c * BL:(c + 1) * BL])}
        for c in range(NCORES)
    ]
    return in_maps


_NC_CACHE = []


def kernel(x, gn_gamma, gn_beta, qkv_w, qkv_b, proj_w, proj_b, trace=False):
    in_maps = prep_inputs(x, gn_gamma, gn_beta, qkv_w, qkv_b, proj_w, proj_b)
    if not _NC_CACHE:
        _NC_CACHE.append(build_bass())
    nc = _NC_CACHE[0]
    res = run_bass_kernel_spmd(nc, in_maps, list(range(NCORES)), trace=trace)
    y = np.stack([res.results[c]["y"] for c in range(NCORES)])
    y = y.reshape(B, C, HH, WW)
    kernel.last_result = res
    return y


# revision 14
# speedup vs baseline: 1.1887x; 1.1887x over previous
"""AttentionBlock (GroupNorm + 4-head self-attention + proj + residual) on 8 trn2 cores.

Sharding: data-parallel over batch (B=16 -> 2 per core). Each core runs the full
block on its 2 batch elements; no collectives.

Device pipeline per batch (all layouts chosen so no on-device transposes are needed):
  - GroupNorm stats via bn_stats/bn_aggr + tiny PE matmuls for the cross-partition
    group combine (gamma/beta folded into the qkv weights on host).
  - Q,K GEMM in [channel, spatial] layout; V^T via a swapped GEMM (stationary=h).
  - Scores computed transposed: S^T[m,n] = K^T Q, so softmax's sum runs over the
    PSUM partition axis, computed by an all-ones stationary matmul that also
    replicates the denominator Z across partitions (no broadcast needed).
  - exp on ACT straight out of PSUM; no max-subtraction (scores bounded ~|8|).
  - AV as O^T[d,n] = sum_m V^T[m,d] expS^T[m,n], 2 heads packed per 128-col array.
  - proj GEMM + residual; biases injected via rank-1 (K=1) matmuls into PSUM.
"""

import numpy as np
from contextlib import ExitStack

import concourse.bass as bass
import concourse.bacc as bacc
import concourse.tile as tile
import concourse.mybir as mybir
from concourse.bass_utils import run_bass_kernel_spmd

F32 = mybir.dt.float32
F32R = mybir.dt.float32r

B, C, HH, WW = 16, 256, 32, 32
N = HH * WW           # 1024 spatial positions
NH = 4                # heads
D = C // NH           # 64 head dim
G = 32                # groups
EPS = 1e-5
NCORES = 8
BL = B // NCORES      # batches per core

USE_F32R = True       # fp32r matmuls: full PE rate; flip to False for exact fp32
MM_DT = F32R if USE_F32R else F32


def _R(ap):
    return ap


def build_bass():
    nc = bacc.Bacc("TRN2", target_bir_lowering=False, debug=False)

    x_d = nc.dram_tensor("x", [BL, C, N], F32, kind="ExternalInput").ap()
    wqk_d = nc.dram_tensor("wqk_t", [2, 128, 512], MM_DT, kind="ExternalInput").ap()
    wv_d = nc.dram_tensor("wv_t", [2, 128, 256], MM_DT, kind="ExternalInput").ap()
    wp_d = nc.dram_tensor("wp_t", [2, 128, 256], MM_DT, kind="ExternalInput").ap()
    bqk_d = nc.dram_tensor("bqk", [4, 128], F32, kind="ExternalInput").ap()
    bv_d = nc.dram_tensor("bv", [1, 256], MM_DT, kind="ExternalInput").ap()
    bp_d = nc.dram_tensor("bp", [1, 256], MM_DT, kind="ExternalInput").ap()
    gmap_d = nc.dram_tensor("gmap", [128, 16], F32, kind="ExternalInput").ap()
    gexp_d = nc.dram_tensor("gexp", [16, 128], F32, kind="ExternalInput").ap()
    y_d = nc.dram_tensor("y", [BL, C, N], F32, kind="ExternalOutput").ap()

    Exp = mybir.ActivationFunctionType.Exp
    mult = mybir.AluOpType.mult
    sub = mybir.AluOpType.subtract
    add = mybir.AluOpType.add

    with tile.TileContext(nc) as tc, ExitStack() as ctx:
        consts = ctx.enter_context(tc.tile_pool(name="consts", bufs=1))
        xpool = ctx.enter_context(tc.tile_pool(name="xp", bufs=1))
        hpool = ctx.enter_context(tc.tile_pool(name="hp", bufs=1))
        qkpool = ctx.enter_context(tc.tile_pool(name="qkp", bufs=1))
        vtpool = ctx.enter_context(tc.tile_pool(name="vtp", bufs=1))
        opool = ctx.enter_context(tc.tile_pool(name="op", bufs=1))
        gnpool = ctx.enter_context(tc.tile_pool(name="gnp", bufs=2))
        expool = ctx.enter_context(tc.tile_pool(name="exp", bufs=3))
        rzpool = ctx.enter_context(tc.tile_pool(name="rzp", bufs=2))
        outpool = ctx.enter_context(tc.tile_pool(name="outp", bufs=2))
        ps_big = ctx.enter_context(tc.tile_pool(name="psb", bufs=3, space="PSUM"))
        ps_o = ctx.enter_context(tc.tile_pool(name="pso", bufs=2, space="PSUM"))
        
        # --- constants / weights into SBUF ---
        wqk_sb = [consts.tile([128, 512], MM_DT, tag=f"wqk{k}", name=f"wqk{k}") for k in range(2)]
        wv_sb = [consts.tile([128, 256], MM_DT, tag=f"wv{k}", name=f"wv{k}") for k in range(2)]
        wp_sb = [consts.tile([128, 256], MM_DT, tag=f"wp{k}", name=f"wp{k}") for k in range(2)]
        for k in range(2):
            nc.sync.dma_start(wqk_sb[k][:], wqk_d[k])
            nc.sync.dma_start(wv_sb[k][:], wv_d[k])
            nc.sync.dma_start(wp_sb[k][:], wp_d[k])
        bqk_sb = consts.tile([128, 4], F32, tag="bqk")
        nc.sync.dma_start(bqk_sb[:], bqk_d.transpose([1, 0]))
        bv_sb = consts.tile([1, 256], MM_DT, tag="bv")
        nc.sync.dma_start(bv_sb[:], bv_d[:])
        bp_sb = consts.tile([1, 256], MM_DT, tag="bp")
        nc.sync.dma_start(bp_sb[:], bp_d[:])
        gmap_sb = consts.tile([128, 16], F32, tag="gmap")
        nc.sync.dma_start(gmap_sb[:], gmap_d[:])
        gexp_sb = consts.tile([16, 128], F32, tag="gexp")
        nc.sync.dma_start(gexp_sb[:], gexp_d[:])
        ones_f32 = consts.tile([128, 512], F32, tag="ones_f32")
        nc.vector.memset(ones_f32[:], 1.0)
        ones_sb = consts.tile([128, 512], MM_DT, tag="ones")
        nc.vector.tensor_copy(ones_sb[:], ones_f32[:])
        # bv broadcast to all partitions via a rank-1 matmul (done once)
        pbv = ps_o.tile([128, 256], F32, tag="o", name="pbv")
        nc.tensor.matmul(pbv[:], lhsT=_R(ones_sb[0:1, 0:128]), rhs=_R(bv_sb[0:1, :]),
                         start=True, stop=True)
        bvb = consts.tile([128, 256], F32, tag="bvb")
        nc.vector.tensor_copy(bvb[:], pbv[:])

        x_sb = [[None] * 2 for _ in range(BL)]
        h_sb = [[None] * 2 for _ in range(BL)]
        qk_sb = [[None] * 4 for _ in range(BL)]
        vt_sb = [[None] * 8 for _ in range(BL)]
        o_sb = [[None] * 2 for _ in range(BL)]

        # ================= Phase A: GroupNorm + QKV GEMMs =================
        for b in range(BL):
            for ct in range(2):
                xt = xpool.tile([128, N], F32, tag=f"x{b}{ct}")
                x_sb[b][ct] = xt
                nc.sync.dma_start(xt[:], x_d[b, ct * 128:(ct + 1) * 128, :])

                # per-channel stats over the 1024 free elems
                bn6 = gnpool.tile([128, 12], F32, tag="bn6")
                nc.vector.bn_stats(bn6[:, 0:6], xt[:, 0:512])
                nc.vector.bn_stats(bn6[:, 6:12], xt[:, 512:1024])
                mv = gnpool.tile([128, 2], F32, tag="mv")  # mean, var per channel
                nc.vector.bn_aggr(mv[:], bn6[:])
                # st2 = [mean, E[x^2]] per channel
                st2 = gnpool.tile([128, 2], F32, tag="st2")
                nc.vector.tensor_copy(st2[:, 0:1], mv[:, 0:1])
                nc.vector.tensor_mul(st2[:, 1:2], mv[:, 0:1], mv[:, 0:1])
                nc.vector.tensor_add(st2[:, 1:2], st2[:, 1:2], mv[:, 1:2])
                # group-combine: [128,2] -> [16,2] (x 1/8) -> expand back [128,2]
                psg = ps_o.tile([16, 2], F32, tag="o", name="psg")
                nc.tensor.matmul(psg[:], lhsT=gmap_sb[:], rhs=st2[:], start=True, stop=True)
                gs = gnpool.tile([16, 2], F32, tag="gs")
                nc.vector.tensor_copy(gs[:], psg[:])
                psc = ps_o.tile([128, 2], F32, tag="o", name="psc")
                nc.tensor.matmul(psc[:], lhsT=gexp_sb[:], rhs=gs[:], start=True, stop=True)
                cs = gnpool.tile([128, 2], F32, tag="cs")  # [mean_g, E[x^2]_g]
                nc.vector.tensor_copy(cs[:], psc[:])
                # inv_std = sqrt(1/(var+eps)); var = E[x^2] - mean^2
                m2 = gnpool.tile([128, 1], F32, tag="m2")
                nc.vector.tensor_mul(m2[:], cs[:, 0:1], cs[:, 0:1])
                ve = gnpool.tile([128, 1], F32, tag="ve")
                nc.vector.tensor_sub(ve[:], cs[:, 1:2], m2[:])
                nc.vector.tensor_scalar_add(ve[:], ve[:], EPS)
                iv = gnpool.tile([128, 1], F32, tag="iv")
                nc.vector.reciprocal(iv[:], ve[:])
                s_t = gnpool.tile([128, 1], F32, tag="s_t")
                nc.scalar.sqrt(s_t[:], iv[:])
                t_t = gnpool.tile([128, 1], F32, tag="t_t")  # mean * inv_std
                nc.vector.tensor_mul(t_t[:], cs[:, 0:1], s_t[:])
                # h = x * s - t  (gamma/beta already folded into W/b on host)
                ht = hpool.tile([128, N], MM_DT, tag=f"h{b}{ct}")
                h_sb[b][ct] = ht
                nc.vector.tensor_scalar(ht[:], xt[:], s_t[:, 0:1], t_t[:, 0:1], mult, sub)

            # Q,K GEMM: out channels ot: 0=q01 1=q23 2=k01 3=k23 (K pre-scaled by 1/8)
            for ot in range(4):
                pq = ps_big.tile([128, N], F32, tag="big")
                for nch in range(2):
                    ns = slice(nch * 512, (nch + 1) * 512)
                    for k in range(2):
                        nc.tensor.matmul(
                            pq[:, ns],
                            lhsT=_R(wqk_sb[k][:, ot * 128:(ot + 1) * 128]),
                            rhs=_R(h_sb[b][k][:, ns]),
                            start=(k == 0), stop=(k == 1),
                        )
                qk = qkpool.tile([128, N], MM_DT, tag=f"qk{b}{ot}")
                qk_sb[b][ot] = qk
                nc.vector.tensor_scalar(qk[:], pq[:], bqk_sb[:, ot:ot + 1], None, add)

            # V^T GEMM: V^T[m, vc] = sum_c h[c,m] WvT[c,vc]  (+ bv via rank-1 matmul)
            for m in range(8):
                pv = ps_o.tile([128, 512], F32, tag="o")
                mc = slice(m * 128, (m + 1) * 128)
                for k in range(2):
                    nc.tensor.matmul(
                        pv[:, 0:256],
                        lhsT=_R(h_sb[b][k][:, mc]),
                        rhs=_R(wv_sb[k][:]),
                        start=(k == 0), stop=(k == 1),
                    )
                # vt layout [128, 512]: per global head h a 128-col block;
                # even h: [V_h | ones], odd h: [ones | V_h].  The ones columns
                # make the AV matmul also emit the softmax denominator Z
                # (replicated over 64 partitions) in the same PSUM bank.
                vt = vtpool.tile([128, 512], MM_DT, tag=f"vt{b}{m}")
                vt_sb[b][m] = vt
                vt4 = vt[:].rearrange("p (a u v d) -> p a u v d", a=2, u=2, v=2)
                pv4 = pv[:, 0:256].rearrange("p (a w d) -> p a w d", a=2, w=2)
                of = ones_f32[:, 0:128].rearrange("p (a d) -> p a d", a=2)
                bvb4 = bvb[:].rearrange("p (a w d) -> p a w d", a=2, w=2)
                nc.vector.tensor_add(vt4[:, :, 0, 0, :], pv4[:, :, 0, :], bvb4[:, :, 0, :])
                nc.vector.tensor_add(vt4[:, :, 1, 1, :], pv4[:, :, 1, :], bvb4[:, :, 1, :])
                nc.vector.tensor_copy(vt4[:, :, 0, 1, :], of[:])
                nc.vector.tensor_copy(vt4[:, :, 1, 0, :], of[:])

        # ================= Phase B: attention, Phase C: proj+residual ======
        for b in range(BL):
            for p in range(2):
                ot = opool.tile([128, N], MM_DT, tag=f"o{b}{p}")
                o_sb[b][p] = ot
                qt = qk_sb[b][p]
                kt = qk_sb[b][2 + p]
                h0, h1 = 2 * p, 2 * p + 1
                for nch in range(2):
                    ns = slice(nch * 512, (nch + 1) * 512)
                    po0 = ps_o.tile([128, 512], F32, tag="o", name="po0")
                    po1 = ps_o.tile([128, 512], F32, tag="o", name="po1")
                    for m in range(8):
                        mc = slice(m * 128, (m + 1) * 128)
                        ps = ps_big.tile([128, N], F32, tag="big")
                        # S^T chunk: [m(128 part), n(512)] x 2 heads (row-packed)
                        nc.tensor.matmul(
                            ps[:, 0:512],
                            lhsT=_R(kt[0:64, mc]), rhs=_R(qt[0:64, ns]),
                            start=True, stop=True,
                        )
                        nc.tensor.matmul(
                            ps[:, 512:1024],
                            lhsT=_R(kt[64:128, mc]), rhs=_R(qt[64:128, ns]),
                            start=True, stop=True,
                        )
                        ex = expool.tile([128, N], MM_DT, tag="ex")
                        nc.scalar.activation(ex[:], ps[:], Exp)
                        first, last = (m == 0), (m == 7)
                        # AV+Z: [V_h0|1] -> O rows 0:64, Zrep rows 64:128
                        nc.tensor.matmul(
                            po0[:],
                            lhsT=_R(vt_sb[b][m][:, 128 * h0:128 * h0 + 128]),
                            rhs=_R(ex[:, 0:512]),
                            start=first, stop=last,
                        )
                        # [1|V_h1] -> Zrep rows 0:64, O rows 64:128
                        nc.tensor.matmul(
                            po1[:],
                            lhsT=_R(vt_sb[b][m][:, 128 * h1:128 * h1 + 128]),
                            rhs=_R(ex[:, 512:1024]),
                            start=first, stop=last,
                        )
                    # head h0: O at rows 0:64; Z sits at rows 64:128 (PSUM can't
                    # DMA and approx-recip breaks off base partition 0, so:
                    # DVE copy out, DMA shift down, recip at base 0, multiply)
                    zc0 = rzpool.tile([128, 512], F32, tag="zc0")
                    nc.vector.tensor_copy(zc0[64:128, :], po0[64:128, :])
                    zs0 = rzpool.tile([64, 512], F32, tag="zs0")
                    nc.sync.dma_start(zs0[:], zc0[64:128, :])
                    rzs0 = rzpool.tile([64, 512], F32, tag="rzs0")
                    nc.vector.reciprocal_approx_fast(rzs0[:], zs0[:])
                    nc.vector.tensor_mul(ot[0:64, ns], po0[0:64, :], rzs0[:])
                    # head h1: O at rows 64:128, 1/Z shifts up
                    rz1 = rzpool.tile([128, 512], F32, tag="rz", name="rz1")
                    nc.vector.reciprocal_approx_fast(rz1[0:64, :], po1[0:64, :])
                    rzs1 = rzpool.tile([128, 512], F32, tag="rzs1")
                    nc.sync.dma_start(rzs1[64:128, :], rz1[0:64, :])
                    nc.vector.tensor_mul(ot[64:128, ns], po1[64:128, :], rzs1[64:128, :])

            # proj + residual + bias
            for ct in range(2):
                pp = ps_big.tile([128, N], F32, tag="big")
                for nch in range(2):
                    ns = slice(nch * 512, (nch + 1) * 512)
                    for k in range(2):
                        nc.tensor.matmul(
                            pp[:, ns],
                            lhsT=_R(wp_sb[k][:, ct * 128:(ct + 1) * 128]),
                            rhs=_R(o_sb[b][k][:, ns]),
                            start=(k == 0), stop=False,
                        )
                    nc.tensor.matmul(
                        pp[:, ns],
                        lhsT=_R(bp_sb[0:1, ct * 128:(ct + 1) * 128]),
                        rhs=_R(ones_sb[0:1, :]),
                        start=False, stop=True,
                    )
                outt = outpool.tile([128, N], F32, tag="out")
                nc.vector.tensor_add(outt[:], pp[:], x_sb[b][ct][:])
                nc.sync.dma_start(y_d[b, ct * 128:(ct + 1) * 128, :], outt[:])

    nc.compile()
    return nc


def prep_inputs(x, gn_gamma, gn_beta, qkv_w, qkv_b, proj_w, proj_b):
    """Host-side weight prep shared by kernel() and the CoreSim test."""
    x = np.ascontiguousarray(np.asarray(x, np.float32)).reshape(B, C, N)
    gn_gamma = np.asarray(gn_gamma, np.float32)
    gn_beta = np.asarray(gn_beta, np.float32)
    qkv_w = np.asarray(qkv_w, np.float32)
    qkv_b = np.asarray(qkv_b, np.float32)
    proj_w = np.asarray(proj_w, np.float32)
    proj_b = np.asarray(proj_b, np.float32)

    # fold GroupNorm affine into the qkv GEMM
    W3 = qkv_w * gn_gamma[None, :]
    b3 = qkv_b + qkv_w @ gn_beta
    W3r = W3.reshape(NH, 3, D, C)
    b3r = b3.reshape(NH, 3, D)
    scale = np.float32(D ** -0.5)
    Wq = W3r[:, 0].reshape(C, C)
    Wk = W3r[:, 1].reshape(C, C) * scale   # fold the attention scale into K
    Wv = W3r[:, 2].reshape(C, C)
    bq = b3r[:, 0].reshape(C)
    bk = b3r[:, 1].reshape(C) * scale
    bv = b3r[:, 2].reshape(C)

    wqk_t = np.ascontiguousarray(
        np.concatenate([Wq, Wk], axis=0).T).reshape(2, 128, 512)
    wv_t = np.ascontiguousarray(Wv.T).reshape(2, 128, 256)
    wp_t = np.ascontiguousarray(proj_w.T).reshape(2, 128, 256)
    bqk = np.concatenate([bq, bk]).reshape(4, 128)

    cidx = np.arange(128)
    gmap = np.zeros((128, 16), np.float32)
    gmap[cidx, cidx // 8] = 1.0 / 8.0
    gexp = np.zeros((16, 128), np.float32)
    gexp[cidx // 8, cidx] = 1.0

    common = {
        "wqk_t": wqk_t.astype(np.float32),
        "wv_t": wv_t.astype(np.float32),
        "wp_t": wp_t.astype(np.float32),
        "bqk": bqk.astype(np.float32),
        "bv": np.ascontiguousarray(bv[None, :], np.float32),
        "bp": np.ascontiguousarray(proj_b[None, :], np.float32),
        "gmap": gmap,
        "gexp": gexp,
    }
    in_maps = [
        {**common, "x": np.ascontiguousarray(x[c * BL:(c + 1) * BL])}
        for c in range(NCORES)
    ]
    return in_maps


_NC_CACHE = []


def kernel(x, gn_gamma, gn_beta, qkv_w, qkv_b, proj_w, proj_b, trace=False):
    in_maps = prep_inputs(x, gn_gamma, gn_beta, qkv_w, qkv_b, proj_w, proj_b)
    if not _NC_CACHE:
        _NC_CACHE.append(build_bass())
    nc = _NC_CACHE[0]
    res = run_bass_kernel_spmd(nc, in_maps, list(range(NCORES)), trace=trace)
    y = np.stack([res.results[c]["y"] for c in range(NCORES)])
    y = y.reshape(B, C, HH, WW)
    kernel.last_result = res
    return y
